# revision 5
# baseline (speedup 1.0000x reference)
"""Causal self-attention (B=2, T=2048, C=1024, H=16) on 8 trn2 NeuronCores.

Sharding: core c = (b, g) with b = c // 4 (batch), g = c % 4 (head-group of 4
heads = 256 dims). Per core:
  1. QKV projection from x[b].T (fp32r matmuls, bias fused into DVE copies):
     Q^T, K^T in [d, t] layout (head-pair tiles), V in [t, d] layout with a
     ones column appended per head (gives softmax denominators for free).
  2. Flash-style attention in S^T = K Q^T layout (no transposes anywhere):
     S^T[k, q] tiles -> exp (ACT, scale=1/8 fused) -> diagonal-block causal
     mask (DVE mul) -> AV accumulation with V-as-lhsT.  Row 64 of the AV
     output is the softmax denominator; normalize via reciprocal +
     gpsimd.partition_broadcast.
  3. AllGather of y^T [256, 2048] within each batch's 4-core group.
  4. Output projection column-sharded: each core computes o^T[e-slice, t]
     for its 256 output columns (uniform program; w_proj slice is input
     data).  Host transposes and concatenates.
"""
import math

import numpy as np

B, T, C, H = 2, 2048, 1024, 16
HD = C // H          # 64 head dim
G = 4                # head-groups (cores per batch)
HPG = H // G         # 4 heads per group
DG = HPG * HD        # 256 dims per group
N_CORES = 8
KC = C // 128        # 8 contraction chunks
NKT = T // 128       # 16 k-tiles
NQC = T // 1024      # 2 q-chunks of 1024 in attention

_NC_CACHE = {}


def _pieces(qs):
    """Split [qs, 1024) into <=512-wide matmul pieces."""
    if qs < 512:
        return [(qs, 512 - qs), (512, 512)]
    return [(qs, 1024 - qs)]


def _build():
    import concourse.bacc as bacc
    import concourse.mybir as mybir
    import concourse.tile as tile

    f32 = mybir.dt.float32
    f32r = mybir.dt.float32r
    Exp = mybir.ActivationFunctionType.Exp

    nc = bacc.Bacc("TRN2", num_devices=N_CORES)

    xT_d = nc.dram_tensor("xT", [C, T], f32r, kind="ExternalInput")
    wq_d = nc.dram_tensor("wq", [C, DG], f32r, kind="ExternalInput")
    wk_d = nc.dram_tensor("wk", [C, DG], f32r, kind="ExternalInput")
    wv_d = nc.dram_tensor("wv", [C, DG], f32r, kind="ExternalInput")
    bq_d = nc.dram_tensor("bq", [2, 128, 1], f32, kind="ExternalInput")
    bk_d = nc.dram_tensor("bk", [2, 128, 1], f32, kind="ExternalInput")
    bv_d = nc.dram_tensor("bv", [1, DG], f32, kind="ExternalInput")
    wp_d = nc.dram_tensor("wpT", [C, DG], f32r, kind="ExternalInput")
    bp_d = nc.dram_tensor("bp", [2, 128, 1], f32, kind="ExternalInput")
    mask_d = nc.dram_tensor("mask", [128, 128], f32r, kind="ExternalInput")
    ones_d = nc.dram_tensor("ones4", [128, HPG, 1], f32r, kind="ExternalInput")
    oT_d = nc.dram_tensor("oT", [DG, T], f32, kind="ExternalOutput")

    with tile.TileContext(nc) as tc:
        with (
            tc.tile_pool(name="persist", bufs=1) as persist,
            tc.tile_pool(name="dram", bufs=1, space="DRAM") as dram,
        ):
            # ---- persistent SBUF ----
            QT = [persist.tile([128, T], f32r, name=f"qt{p}") for p in range(2)]
            KT = [persist.tile([128, T], f32r, name=f"kt{p}") for p in range(2)]
            V1 = [persist.tile([128, HPG * (HD + 1)], f32r, name=f"v{m}")
                  for m in range(NKT)]
            yT = [persist.tile([128, T], f32r, name=f"yt{p}") for p in range(2)]
            wpT_sb = [persist.tile([128, DG], f32r, name=f"wp{k}")
                      for k in range(KC)]
            mask_sb = persist.tile([128, 128], f32r, name="mask_sb")
            bq_sb = [persist.tile([128, 1], f32, name=f"bq{j}") for j in range(2)]
            bk_sb = [persist.tile([128, 1], f32, name=f"bk{j}") for j in range(2)]
            bp_sb = [persist.tile([128, 1], f32, name=f"bp{j}") for j in range(2)]
            bv_row = persist.tile([1, DG], f32, name="bv_row")
            bv_bc = persist.tile([128, DG], f32, name="bv_bc")

            nc.sync.dma_start(mask_sb[:], mask_d[:])
            for j in range(2):
                nc.sync.dma_start(bq_sb[j][:], bq_d[j])
                nc.sync.dma_start(bk_sb[j][:], bk_d[j])
                nc.sync.dma_start(bp_sb[j][:], bp_d[j])
            nc.sync.dma_start(bv_row[:], bv_d[:])
            nc.gpsimd.partition_broadcast(bv_bc[:], bv_row[:])
            for k in range(KC):
                nc.sync.dma_start(wpT_sb[k][:], wp_d[128 * k:128 * (k + 1), :])

            y_loc = dram.tile([DG, T], f32r, name="y_loc")
            y_full = dram.tile([C, T], f32r, name="y_full")

            # ================= phase 1: QKV =================
            with (
                tc.tile_pool(name="xp", bufs=1) as xp,
                tc.tile_pool(name="wp_s", bufs=1) as wp_s,
                tc.tile_pool(name="qkvps", bufs=1, space="PSUM") as qkvps,
            ):
                xT_sb = []
                for k in range(KC):
                    xt = xp.tile([128, T], f32r, name=f"x{k}")
                    nc.sync.dma_start(xt[:], xT_d[128 * k:128 * (k + 1), :])
                    xT_sb.append(xt)
                wv_sb = []
                for k in range(KC):
                    wvt = wp_s.tile([128, DG], f32r, name=f"wv{k}")
                    nc.sync.dma_start(wvt[:], wv_d[128 * k:128 * (k + 1), :])
                    wv_sb.append(wvt)

                # Q then K: psum [2 jh][4 t4] accumulated over kc
                for sel in range(2):
                    dst = QT if sel == 0 else KT
                    wdram = wq_d if sel == 0 else wk_d
                    bcol = bq_sb if sel == 0 else bk_sb
                    ps = [[qkvps.tile([128, 512], f32, tag="qkvps", bufs=8,
                                      name=f"ps{sel}_{jh}_{t4}")
                           for t4 in range(4)] for jh in range(2)]
                    for kc in range(KC):
                        wt = wp_s.tile([128, DG], f32r, tag="wqk", bufs=3,
                                       name=f"w{sel}_{kc}")
                        nc.sync.dma_start(
                            wt[:], wdram[128 * kc:128 * (kc + 1), :])
                        for jh in range(2):
                            for t4 in range(4):
                                nc.tensor.matmul(
                                    ps[jh][t4][:],
                                    wt[:, 128 * jh:128 * (jh + 1)],
                                    xT_sb[kc][:, 512 * t4:512 * (t4 + 1)],
                                    start=(kc == 0), stop=(kc == KC - 1))
                    for jh in range(2):
                        for t4 in range(4):
                            nc.vector.tensor_scalar_add(
                                dst[jh][:, 512 * t4:512 * (t4 + 1)],
                                ps[jh][t4][:], bcol[jh][:])

                # V: [t, d] layout, heads at stride 65 with ones column
                for mt in range(NKT):
                    psv = qkvps.tile([128, DG], f32, tag="qkvps", bufs=8,
                                     name=f"psv{mt}")
                    for kc in range(KC):
                        nc.tensor.matmul(
                            psv[:],
                            xT_sb[kc][:, 128 * mt:128 * (mt + 1)],
                            wv_sb[kc][:],
                            start=(kc == 0), stop=(kc == KC - 1))
                    vv = V1[mt].rearrange("p (h x) -> p h x", h=HPG)
                    nc.vector.tensor_add(
                        vv[:, :, 0:HD],
                        psv.rearrange("p (h x) -> p h x", h=HPG),
                        bv_bc.rearrange("p (h x) -> p h x", h=HPG))
                    nc.sync.dma_start(vv[:, :, HD:HD + 1], ones_d[:])

            # ================= phase 2: attention =================
            with (
                tc.tile_pool(name="aps", bufs=1, space="PSUM") as aps,
                tc.tile_pool(name="ppool", bufs=1) as ppool,
                tc.tile_pool(name="npool", bufs=1) as npool,
            ):
                for p in range(2):
                    for cq in range(NQC):
                        yps = [aps.tile([HD + 1, 1024], f32, tag=f"y{X}",
                                        bufs=1, name=f"y_{p}_{cq}_{X}")
                               for X in range(2)]
                        nkt = 8 * (cq + 1)
                        for kt in range(nkt):
                            qs = max(0, 128 * kt - 1024 * cq)
                            S = aps.tile([128, 2048], f32, tag="s", bufs=1,
                                         name=f"s_{p}_{cq}_{kt}")
                            for X in range(2):
                                for (a, n) in _pieces(qs):
                                    nc.tensor.matmul(
                                        S[:, 1024 * X + a:1024 * X + a + n],
                                        KT[p][64 * X:64 * (X + 1),
                                              128 * kt:128 * (kt + 1)],
                                        QT[p][64 * X:64 * (X + 1),
                                              1024 * cq + a:1024 * cq + a + n],
                                        start=True, stop=True)
                            Pt = ppool.tile([128, 2048], f32r, tag="p", bufs=2,
                                            name=f"p_{p}_{cq}_{kt}")
                            nc.scalar.activation(
                                out=Pt.rearrange("pp (x q) -> pp x q",
                                                 x=2)[:, :, qs:1024],
                                in_=S.rearrange("pp (x q) -> pp x q",
                                                x=2)[:, :, qs:1024],
                                func=Exp, scale=1.0 / math.sqrt(HD))
                            if kt >= 8 * cq:  # diagonal block
                                for X in range(2):
                                    nc.vector.tensor_mul(
                                        Pt[:, 1024 * X + qs:1024 * X + qs + 128],
                                        Pt[:, 1024 * X + qs:1024 * X + qs + 128],
                                        mask_sb[:])
                            for X in range(2):
                                h = 2 * p + X
                                for (a, n) in _pieces(qs):
                                    nc.tensor.matmul(
                                        yps[X][:, a:a + n],
                                        V1[kt][:, (HD + 1) * h:
                                               (HD + 1) * (h + 1)],
                                        Pt[:, 1024 * X + a:1024 * X + a + n],
                                        start=(kt == 0), stop=(kt == nkt - 1))
                        for X in range(2):
                            rec0 = npool.tile([1, 1024], f32, tag="rec0",
                                              bufs=2, name=f"r0_{p}_{cq}_{X}")
                            nc.vector.reciprocal(rec0[:], yps[X][HD:HD + 1, :])
                            bcx = npool.tile([HD, 1024], f32, tag="bc",
                                             bufs=2, name=f"bc_{p}_{cq}_{X}")
                            nc.gpsimd.partition_broadcast(bcx[:], rec0[:])
                            nc.vector.tensor_mul(
                                yT[p][64 * X:64 * (X + 1),
                                      1024 * cq:1024 * (cq + 1)],
                                yps[X][0:HD, :], bcx[:])

            # ================= phase 3: AllGather =================
            import concourse.mybir as mybir_
            for p in range(2):
                nc.sync.dma_start(y_loc[128 * p:128 * (p + 1), :], yT[p][:])
            nc.gpsimd.collective_compute(
                "AllGather",
                mybir_.AluOpType.bypass,
                replica_groups=[[0, 1, 2, 3], [4, 5, 6, 7]],
                ins=[y_loc[:].opt()],
                outs=[y_full[:].opt()],
            )

            # ================= phase 4: projection (o^T) =================
            with (
                tc.tile_pool(name="yfp", bufs=1) as yfp,
                tc.tile_pool(name="pps", bufs=1, space="PSUM") as pps,
                tc.tile_pool(name="otp", bufs=1) as otp,
            ):
                yf_sb = []
                for kd in range(KC):
                    yf = yfp.tile([128, T], f32r, name=f"yf{kd}")
                    nc.sync.dma_start(yf[:], y_full[128 * kd:128 * (kd + 1), :])
                    yf_sb.append(yf)
                for eh in range(2):
                    for t4 in range(4):
                        po = pps.tile([128, 512], f32, tag="po", bufs=4,
                                      name=f"po_{eh}_{t4}")
                        for kd in range(KC):
                            nc.tensor.matmul(
                                po[:],
                                wpT_sb[kd][:, 128 * eh:128 * (eh + 1)],
                                yf_sb[kd][:, 512 * t4:512 * (t4 + 1)],
                                start=(kd == 0), stop=(kd == KC - 1))
                        ot = otp.tile([128, 512], f32, tag="ot", bufs=4,
                                      name=f"ot_{eh}_{t4}")
                        nc.vector.tensor_scalar_add(ot[:], po[:], bp_sb[eh][:])
                        nc.sync.dma_start(
                            oT_d[128 * eh:128 * (eh + 1),
                                 512 * t4:512 * (t4 + 1)], ot[:])

    nc.finalize()
    return nc


def _get_nc():
    if "nc" not in _NC_CACHE:
        _NC_CACHE["nc"] = _build()
    return _NC_CACHE["nc"]


def kernel(x, w_attn, b_attn, w_proj, b_proj):
    from concourse.bass_utils import run_bass_kernel_spmd

    x = np.asarray(x, dtype=np.float32)
    w_attn = np.asarray(w_attn, dtype=np.float32)
    b_attn = np.asarray(b_attn, dtype=np.float32)
    w_proj = np.asarray(w_proj, dtype=np.float32)
    b_proj = np.asarray(b_proj, dtype=np.float32)

    mask = np.triu(np.ones((128, 128), dtype=np.float32)).copy()

    in_maps = []
    for c in range(N_CORES):
        b, g = divmod(c, G)
        lo = DG * g
        in_maps.append({
            "xT": np.ascontiguousarray(x[b].T),
            "wq": np.ascontiguousarray(w_attn[lo:lo + DG, :].T),
            "wk": np.ascontiguousarray(w_attn[C + lo:C + lo + DG, :].T),
            "wv": np.ascontiguousarray(w_attn[2 * C + lo:2 * C + lo + DG, :].T),
            "bq": np.ascontiguousarray(b_attn[lo:lo + DG].reshape(2, 128, 1)),
            "bk": np.ascontiguousarray(
                b_attn[C + lo:C + lo + DG].reshape(2, 128, 1)),
            "bv": np.ascontiguousarray(
                b_attn[2 * C + lo:2 * C + lo + DG].reshape(1, DG)),
            "wpT": np.ascontiguousarray(w_proj[lo:lo + DG, :].T),
            "bp": np.ascontiguousarray(b_proj[lo:lo + DG].reshape(2, 128, 1)),
            "mask": mask,
            "ones4": np.ones((128, HPG, 1), dtype=np.float32),
        })

    global _last_in_maps
    _last_in_maps = in_maps

    nc = _get_nc()
    res = run_bass_kernel_spmd(nc, in_maps, list(range(N_CORES)))

    out = np.empty((B, T, C), dtype=np.float32)
    for c in range(N_CORES):
        b, g = divmod(c, G)
        out[b, :, DG * g:DG * (g + 1)] = res.results[c]["oT"].T
    return out


# revision 11
# speedup vs baseline: 1.2722x; 1.2722x over previous
"""Causal self-attention (B=2, T=2048, C=1024, H=16) on 8 trn2 NeuronCores.

Sharding: core c = (b, g) with b = c // 4 (batch), g = c % 4 (head-group of 4
heads = 256 dims). Per core:
  1. QKV projection from x[b].T (fp32r matmuls, bias fused into DVE copies):
     Q^T, K^T in [d, t] layout (head-pair tiles), V in [t, d] layout with a
     ones column appended per head (gives softmax denominators for free).
  2. Flash-style attention in S^T = K Q^T layout (no transposes anywhere):
     S^T[k, q] tiles -> exp (ACT, scale=1/8 fused) -> diagonal-block causal
     mask (DVE mul) -> AV accumulation with V-as-lhsT.  Row 64 of the AV
     output is the softmax denominator; normalize via reciprocal +
     gpsimd.partition_broadcast.
  3. AllGather of y^T [256, 2048] within each batch's 4-core group.
  4. Output projection column-sharded: each core computes o^T[e-slice, t]
     for its 256 output columns (uniform program; w_proj slice is input
     data).  Host transposes and concatenates.
"""
import math

import numpy as np

B, T, C, H = 2, 2048, 1024, 16
HD = C // H          # 64 head dim
G = 4                # head-groups (cores per batch)
HPG = H // G         # 4 heads per group
DG = HPG * HD        # 256 dims per group
N_CORES = 8
KC = C // 128        # 8 contraction chunks
NKT = T // 128       # 16 k-tiles
NQC = T // 1024      # 2 q-chunks of 1024 in attention

_NC_CACHE = {}


def _pieces(qs):
    """Split [qs, 1024) into <=512-wide matmul pieces."""
    if qs < 512:
        return [(qs, 512 - qs), (512, 512)]
    return [(qs, 1024 - qs)]


def _build():
    import concourse.bacc as bacc
    import concourse.mybir as mybir
    import concourse.tile as tile

    f32 = mybir.dt.float32
    f32r = mybir.dt.float32r
    Exp = mybir.ActivationFunctionType.Exp

    nc = bacc.Bacc("TRN2", num_devices=N_CORES)

    xT_d = nc.dram_tensor("xT", [C, T], f32r, kind="ExternalInput")
    wq_d = nc.dram_tensor("wq", [C, DG], f32r, kind="ExternalInput")
    wk_d = nc.dram_tensor("wk", [C, DG], f32r, kind="ExternalInput")
    wv_d = nc.dram_tensor("wv", [C, DG], f32r, kind="ExternalInput")
    bq_d = nc.dram_tensor("bq", [2, 128, 1], f32, kind="ExternalInput")
    bk_d = nc.dram_tensor("bk", [2, 128, 1], f32, kind="ExternalInput")
    bv_d = nc.dram_tensor("bv", [1, DG], f32, kind="ExternalInput")
    wp_d = nc.dram_tensor("wpT", [C, DG], f32r, kind="ExternalInput")
    bp_d = nc.dram_tensor("bp", [2, 128, 1], f32, kind="ExternalInput")
    mask_d = nc.dram_tensor("mask", [128, 128], f32r, kind="ExternalInput")
    ones_d = nc.dram_tensor("ones4", [128, HPG, 1], f32r, kind="ExternalInput")
    oT_d = nc.dram_tensor("oT", [DG, T], f32, kind="ExternalOutput")

    with tile.TileContext(nc) as tc:
        with (
            tc.tile_pool(name="persist", bufs=1) as persist,
            tc.tile_pool(name="dram", bufs=1, space="DRAM") as dram,
        ):
            # ---- persistent SBUF ----
            QT = [persist.tile([128, T], f32r, name=f"qt{p}") for p in range(2)]
            KT = [persist.tile([128, T], f32r, name=f"kt{p}") for p in range(2)]
            V1 = [persist.tile([128, HPG * (HD + 1)], f32r, name=f"v{m}")
                  for m in range(NKT)]
            yT = [persist.tile([128, T], f32r, name=f"yt{p}") for p in range(2)]
            wpT_sb = [persist.tile([128, DG], f32r, name=f"wp{k}")
                      for k in range(KC)]
            mask_sb = persist.tile([128, 128], f32r, name="mask_sb")
            bq_sb = [persist.tile([128, 1], f32, name=f"bq{j}") for j in range(2)]
            bk_sb = [persist.tile([128, 1], f32, name=f"bk{j}") for j in range(2)]
            bp_sb = [persist.tile([128, 1], f32, name=f"bp{j}") for j in range(2)]
            bv_row = persist.tile([1, DG], f32, name="bv_row")
            bv_bc = persist.tile([128, DG], f32, name="bv_bc")

            nc.sync.dma_start(mask_sb[:], mask_d[:])
            for j in range(2):
                nc.sync.dma_start(bq_sb[j][:], bq_d[j])
                nc.sync.dma_start(bk_sb[j][:], bk_d[j])
                nc.sync.dma_start(bp_sb[j][:], bp_d[j])
            nc.sync.dma_start(bv_row[:], bv_d[:])
            nc.gpsimd.partition_broadcast(bv_bc[:], bv_row[:])

            # quarter buffers for the pipelined AllGather (one per (cq, p))
            yq_in = [[dram.tile([128, 1024], f32r, name=f"yqi_{cq}_{p}")
                      for p in range(2)] for cq in range(NQC)]
            yq_full = [[dram.tile([512, 1024], f32r, name=f"yqf_{cq}_{p}")
                        for p in range(2)] for cq in range(NQC)]

            # ================= phase 1: QKV =================
            with (
                tc.tile_pool(name="xp", bufs=1) as xp,
                tc.tile_pool(name="wp_s", bufs=1) as wp_s,
                tc.tile_pool(name="qkvps", bufs=1, space="PSUM") as qkvps,
            ):
                xT_sb = []
                for k in range(KC):
                    xt = xp.tile([128, T], f32r, name=f"x{k}")
                    nc.sync.dma_start(xt[:], xT_d[128 * k:128 * (k + 1), :])
                    xT_sb.append(xt)
                # Q then K: psum [2 jh][4 t4] accumulated over kc
                for sel in range(2):
                    dst = QT if sel == 0 else KT
                    wdram = wq_d if sel == 0 else wk_d
                    bcol = bq_sb if sel == 0 else bk_sb
                    ps = [[qkvps.tile([128, 512], f32, tag="qkvps", bufs=8,
                                      name=f"ps{sel}_{jh}_{t4}")
                           for t4 in range(4)] for jh in range(2)]
                    for kc in range(KC):
                        wt = wp_s.tile([128, DG], f32r, tag="wqk", bufs=3,
                                       name=f"w{sel}_{kc}")
                        nc.sync.dma_start(
                            wt[:], wdram[128 * kc:128 * (kc + 1), :])
                        for jh in range(2):
                            for t4 in range(4):
                                nc.tensor.matmul(
                                    ps[jh][t4][:],
                                    wt[:, 128 * jh:128 * (jh + 1)],
                                    xT_sb[kc][:, 512 * t4:512 * (t4 + 1)],
                                    start=(kc == 0), stop=(kc == KC - 1))
                    for jh in range(2):
                        for t4 in range(4):
                            nc.vector.tensor_scalar_add(
                                dst[jh][:, 512 * t4:512 * (t4 + 1)],
                                ps[jh][t4][:], bcol[jh][:])

                # V: [t, d] layout, heads at stride 65 with ones column
                wv_sb = []
                for k in range(KC):
                    wvt = wp_s.tile([128, DG], f32r, name=f"wv{k}")
                    nc.sync.dma_start(wvt[:], wv_d[128 * k:128 * (k + 1), :])
                    wv_sb.append(wvt)
                for mt in range(NKT):
                    psv = qkvps.tile([128, DG], f32, tag="qkvps", bufs=8,
                                     name=f"psv{mt}")
                    for kc in range(KC):
                        nc.tensor.matmul(
                            psv[:],
                            xT_sb[kc][:, 128 * mt:128 * (mt + 1)],
                            wv_sb[kc][:],
                            start=(kc == 0), stop=(kc == KC - 1))
                    vv = V1[mt].rearrange("p (h x) -> p h x", h=HPG)
                    nc.vector.tensor_add(
                        vv[:, :, 0:HD],
                        psv.rearrange("p (h x) -> p h x", h=HPG),
                        bv_bc.rearrange("p (h x) -> p h x", h=HPG))
                    nc.sync.dma_start(vv[:, :, HD:HD + 1], ones_d[:])

            # ================= phase 2: attention =================
            with (
                tc.tile_pool(name="aps", bufs=1, space="PSUM") as aps,
                tc.tile_pool(name="ppool", bufs=1) as ppool,
                tc.tile_pool(name="npool", bufs=1) as npool,
            ):
                for cq in range(NQC):
                    for p in range(2):
                        yps = [aps.tile([HD + 1, 1024], f32, tag=f"y{X}",
                                        bufs=1, name=f"y_{p}_{cq}_{X}")
                               for X in range(2)]
                        nkt = 8 * (cq + 1)
                        for kt in range(nkt):
                            qs = max(0, 128 * kt - 1024 * cq)
                            S = aps.tile([128, 2048], f32, tag="s", bufs=1,
                                         name=f"s_{p}_{cq}_{kt}")
                            for X in range(2):
                                for (a, n) in _pieces(qs):
                                    nc.tensor.matmul(
                                        S[:, 1024 * X + a:1024 * X + a + n],
                                        KT[p][64 * X:64 * (X + 1),
                                              128 * kt:128 * (kt + 1)],
                                        QT[p][64 * X:64 * (X + 1),
                                              1024 * cq + a:1024 * cq + a + n],
                                        start=True, stop=True)
                            Pt = ppool.tile([128, 2048], f32r, tag="p", bufs=2,
                                            name=f"p_{p}_{cq}_{kt}")
                            nc.scalar.activation(
                                out=Pt.rearrange("pp (x q) -> pp x q",
                                                 x=2)[:, :, qs:1024],
                                in_=S.rearrange("pp (x q) -> pp x q",
                                                x=2)[:, :, qs:1024],
                                func=Exp, scale=1.0 / math.sqrt(HD))
                            if kt >= 8 * cq:  # diagonal block
                                for X in range(2):
                                    nc.vector.tensor_mul(
                                        Pt[:, 1024 * X + qs:1024 * X + qs + 128],
                                        Pt[:, 1024 * X + qs:1024 * X + qs + 128],
                                        mask_sb[:])
                            for X in range(2):
                                h = 2 * p + X
                                for (a, n) in _pieces(qs):
                                    nc.tensor.matmul(
                                        yps[X][:, a:a + n],
                                        V1[kt][:, (HD + 1) * h:
                                               (HD + 1) * (h + 1)],
                                        Pt[:, 1024 * X + a:1024 * X + a + n],
                                        start=(kt == 0), stop=(kt == nkt - 1))
                        # normalization moved off PSUM: copy y' out, free
                        # the banks, then recip/broadcast/mul from SBUF
                        for X in range(2):
                            ycp = npool.tile([HD + 1, 1024], f32, tag="ycp",
                                             bufs=4, name=f"yc_{p}_{cq}_{X}")
                            nc.vector.tensor_copy(ycp[:], yps[X][:])
                            rec0 = npool.tile([1, 1024], f32, tag="rec0",
                                              bufs=2, name=f"r0_{p}_{cq}_{X}")
                            nc.vector.reciprocal(rec0[:], ycp[HD:HD + 1, :])
                            bcx = npool.tile([HD, 1024], f32, tag="bc",
                                             bufs=2, name=f"bc_{p}_{cq}_{X}")
                            nc.gpsimd.partition_broadcast(bcx[:], rec0[:])
                            nc.vector.tensor_mul(
                                yT[p][64 * X:64 * (X + 1),
                                      1024 * cq:1024 * (cq + 1)],
                                ycp[0:HD, :], bcx[:])
                        # quarter AllGather, pipelined with remaining work
                        nc.sync.dma_start(
                            yq_in[cq][p][:],
                            yT[p][:, 1024 * cq:1024 * (cq + 1)])
                        nc.gpsimd.collective_compute(
                            "AllGather",
                            mybir.AluOpType.bypass,
                            replica_groups=[[0, 1, 2, 3], [4, 5, 6, 7]],
                            ins=[yq_in[cq][p][:].opt()],
                            outs=[yq_full[cq][p][:].opt()],
                        )

            # ================= phase 4: projection (o^T) =================
            # yq_full[cq][p] rows: 128*g + 64*X + dd for head 4g+2p+X, so
            # contraction chunk kd (d in [128kd, 128kd+128)) lives in buffer
            # (cq, p=kd%2) rows [128*(kd//2) : 128*(kd//2)+128).
            with (
                tc.tile_pool(name="yfp", bufs=1) as yfp,
                tc.tile_pool(name="pps", bufs=1, space="PSUM") as pps,
                tc.tile_pool(name="otp", bufs=1) as otp,
            ):
                for k in range(KC):
                    nc.sync.dma_start(wpT_sb[k][:],
                                      wp_d[128 * k:128 * (k + 1), :])
                for c01 in range(NQC):
                    yf_sb = []
                    for kd in range(KC):
                        yf = yfp.tile([128, 1024], f32r, tag="yf", bufs=16,
                                      name=f"yf_{c01}_{kd}")
                        g2, p2 = divmod(kd, 2)
                        nc.sync.dma_start(
                            yf[:],
                            yq_full[c01][p2][128 * g2:128 * (g2 + 1), :])
                        yf_sb.append(yf)
                    for th in range(2):
                        t4 = 2 * c01 + th
                        for eh in range(2):
                            po = pps.tile([128, 512], f32, tag="po", bufs=4,
                                          name=f"po_{eh}_{t4}")
                            for kd in range(KC):
                                nc.tensor.matmul(
                                    po[:],
                                    wpT_sb[kd][:, 128 * eh:128 * (eh + 1)],
                                    yf_sb[kd][:, 512 * th:512 * (th + 1)],
                                    start=(kd == 0), stop=(kd == KC - 1))
                            ot = otp.tile([128, 512], f32, tag="ot", bufs=4,
                                          name=f"ot_{eh}_{t4}")
                            nc.vector.tensor_scalar_add(ot[:], po[:],
                                                        bp_sb[eh][:])
                            nc.sync.dma_start(
                                oT_d[128 * eh:128 * (eh + 1),
                                     512 * t4:512 * (t4 + 1)], ot[:])

    nc.finalize()
    return nc


def _get_nc():
    if "nc" not in _NC_CACHE:
        _NC_CACHE["nc"] = _build()
    return _NC_CACHE["nc"]


def kernel(x, w_attn, b_attn, w_proj, b_proj):
    from concourse.bass_utils import run_bass_kernel_spmd

    x = np.asarray(x, dtype=np.float32)
    w_attn = np.asarray(w_attn, dtype=np.float32)
    b_attn = np.asarray(b_attn, dtype=np.float32)
    w_proj = np.asarray(w_proj, dtype=np.float32)
    b_proj = np.asarray(b_proj, dtype=np.float32)

    mask = np.triu(np.ones((128, 128), dtype=np.float32)).copy()

    in_maps = []
    for c in range(N_CORES):
        b, g = divmod(c, G)
        lo = DG * g
        in_maps.append({
            "xT": np.ascontiguousarray(x[b].T),
            "wq": np.ascontiguousarray(w_attn[lo:lo + DG, :].T),
            "wk": np.ascontiguousarray(w_attn[C + lo:C + lo + DG, :].T),
            "wv": np.ascontiguousarray(w_attn[2 * C + lo:2 * C + lo + DG, :].T),
            "bq": np.ascontiguousarray(b_attn[lo:lo + DG].reshape(2, 128, 1)),
            "bk": np.ascontiguousarray(
                b_attn[C + lo:C + lo + DG].reshape(2, 128, 1)),
            "bv": np.ascontiguousarray(
                b_attn[2 * C + lo:2 * C + lo + DG].reshape(1, DG)),
            "wpT": np.ascontiguousarray(w_proj[lo:lo + DG, :].T),
            "bp": np.ascontiguousarray(b_proj[lo:lo + DG].reshape(2, 128, 1)),
            "mask": mask,
            "ones4": np.ones((128, HPG, 1), dtype=np.float32),
        })

    global _last_in_maps
    _last_in_maps = in_maps

    nc = _get_nc()
    res = run_bass_kernel_spmd(nc, in_maps, list(range(N_CORES)))

    out = np.empty((B, T, C), dtype=np.float32)
    for c in range(N_CORES):
        b, g = divmod(c, G)
        out[b, :, DG * g:DG * (g + 1)] = res.results[c]["oT"].T
    return out
